# revision 11
# baseline (speedup 1.0000x reference)
"""Trainium2 Bass kernel for the mixed low-rank-expert DCN-v2 block (nn_DCN_51539607711).

Reference math (L=3 layers, E=4 experts, D=512, R=64, B=16384):
  x_{l+1} = sum_e x0 * (tanh(tanh(x_l V_e) C_e) U_e^T + b) * gate_e + x_l
The gate softmaxes a size-1 axis == 1.0 exactly and bias is zero, so the
recurrence telescopes:  x_{l+1} = x0 * (1 + sum_{i<=l} A_i),
  A_i = sum_e U_e tanh(C_e^T tanh(V_e^T x_i)).

v6 design (HAM-aware rotating pipeline):
 - v/cv stages in bf16 (fp8 without DoubleRow runs at bf16 speed on the PE,
   so fp8 there bought only error).  ucv stage keeps e4m3 DoubleRow
   (K=256 in one pass).  End-to-end numpy-sim rel err 0.0128.
 - Single input tensor xq = x^T bf16.  The xl update is
   xl' = (s + SU) * xq  (= SU * x_{l+1}); the 1/SU is folded into the
   V weights for layers l>0 (V'_l = V_l / SU), and the host divides the
   output by SU.  No separate x0s tensor, no on-device rescales.
 - 3-span rotating pipeline: tasks = (span, layer) over NB=256-column
   spans, blocks [[0,1],[2,3,4],[5,6,7]].  PE issue order per step i:
   ucv(t-2), cv(t-1), v(t) - every stage trails its producer by ~2 tasks
   of PE work, so the PE never stalls on the tanh/xl-update chain and the
   HAM clock gate stays at 2.4 GHz.
 - PSUM: s accumulators [P,KC,NB] = 2 banks x 3 in-flight spans (pool
   bufs=3) + 2 shared transient banks for vps/cps (pool bufs=2) = 8.
 - PE pre-warm: dummy matmuls on a zeroed SBUF tile run during the input
   DMA window so the HAM un-throttles before real work arrives.
 - Input DMAs split across sync (xq column blocks) and gpsimd (weights)
   queues so issue does not serialize; output written span-major
   ([p][span][chunk][col] -> 2KB contiguous lines per partition).

Distribution: pure data-parallel over B across 8 cores, weights replicated,
activations feature-major ([D, B]), zero on-device transposes.
"""

import numpy as np
import ml_dtypes

import concourse.bacc as bacc
import concourse.tile as tile
from concourse import mybir
from concourse.bass_utils import run_bass_kernel_spmd

L, E, D, R, B = 3, 4, 512, 64, 16384
NCORES = 8
BC = B // NCORES          # batch columns per core (2048)
NB = 256                  # span width
P = 128
KC = D // P               # feature chunks (4)
NPAIR = E // 2            # expert pairs (2)
NSP = BC // NB            # spans per core (8)

SU = 64.0                 # U-scale; folded into V (l>0) and host unscale

F32 = mybir.dt.float32
BF16 = mybir.dt.bfloat16
F8E4 = mybir.dt.float8e4
DR = mybir.MatmulPerfMode.DoubleRow
bf16 = ml_dtypes.bfloat16
f8e4 = ml_dtypes.float8_e4m3

VW_COLS = L * NPAIR * KC * P             # l, pair, chunk, m   (bf16)
UW_COLS = L * KC * 2 * P                 # l, m, plane, mm     (e4m3)
CW_COLS = L * NPAIR * P                  # l, pair, m          (bf16)

BLOCKS = [[0, 1], [2, 3, 4], [5, 6, 7]]
N_WARM = 10               # pre-warm dummy matmuls (256 cols each)

_CACHE = {}


def _build_nc(bc=BC):
    nc = bacc.Bacc("TRN2", target_bir_lowering=False, debug=False,
                   num_devices=NCORES)

    xq_d = nc.dram_tensor("xq", [D, bc], BF16, kind="ExternalInput")
    vw_d = nc.dram_tensor("vw", [P, VW_COLS], BF16, kind="ExternalInput")
    uw_d = nc.dram_tensor("uw", [P, UW_COLS], F8E4, kind="ExternalInput")
    cw_d = nc.dram_tensor("cw", [P, CW_COLS], BF16, kind="ExternalInput")
    out_d = nc.dram_tensor("out_s", [P, NSP * KC * NB], BF16,
                           kind="ExternalOutput")

    out_v = out_d[:].rearrange("p (s m b) -> p s m b", s=NSP, m=KC)

    Tanh = mybir.ActivationFunctionType.Tanh
    ADD = mybir.AluOpType.add
    MULT = mybir.AluOpType.mult

    tasks = [(sp, l) for blk in BLOCKS for l in range(L) for sp in blk]
    # rotation order within each block: (sp0,l0)(sp1,l0)..(sp0,l1)..
    tasks = []
    for blk in BLOCKS:
        for l in range(L):
            for sp in blk:
                tasks.append((sp, l))
    T = len(tasks)

    with tile.TileContext(nc) as tc:
        with (
            tc.tile_pool(name="wpool", bufs=1) as wpool,
            tc.tile_pool(name="xpool", bufs=1) as xpool,
            tc.tile_pool(name="xl_pool", bufs=4) as xl_pool,
            tc.tile_pool(name="vt_pool", bufs=3) as vt_pool,
            tc.tile_pool(name="cvt_pool", bufs=3) as cvt_pool,
            tc.tile_pool(name="ot_pool", bufs=2) as ot_pool,
            tc.tile_pool(name="warm_pool", bufs=1) as warm_pool,
            tc.tile_pool(name="psum_s", bufs=3, space="PSUM") as psum_s,
            tc.tile_pool(name="psum_t", bufs=2, space="PSUM") as psum_t,
        ):
            xq_s = xpool.tile([P, KC, bc], BF16)
            vw_s = wpool.tile([P, VW_COLS], BF16)
            uw_s = wpool.tile([P, UW_COLS], F8E4)
            cw_s = wpool.tile([P, CW_COLS], BF16)

            xq_v = xq_d[:].rearrange("(k p) b -> p k b", p=P)

            # ---- PE pre-warm: dummy matmuls on zeroed SBUF, no DMA deps.
            warm_w = warm_pool.tile([P, NB], BF16)
            nc.gpsimd.memset(warm_w[:], 0.0)
            warm_ps = psum_t.tile([P, NPAIR, NB], F32, name="warm", tag="t")
            for _ in range(N_WARM):
                nc.tensor.matmul(warm_ps[:, 0, :], warm_w[:, 0:P], warm_w[:],
                                 start=True, stop=True,
                                 skip_group_check=True)

            # ---- input DMAs: weights early on gpsimd (vw_l0 first, then the
            # small cw, then uw, then vw_l12); xq column blocks on sync.
            LW = VW_COLS // L
            XB = 512
            nc.sync.dma_start(xq_s[:, :, 0:XB], xq_v[:, :, 0:XB])
            nc.gpsimd.dma_start(vw_s[:, 0:LW], vw_d[:, 0:LW])
            nc.gpsimd.dma_start(cw_s[:], cw_d[:])
            nc.gpsimd.dma_start(uw_s[:], uw_d[:])
            nc.gpsimd.dma_start(vw_s[:, LW:], vw_d[:, LW:])
            for i in range(1, bc // XB):
                nc.sync.dma_start(xq_s[:, :, i * XB:(i + 1) * XB],
                                  xq_v[:, :, i * XB:(i + 1) * XB])

            vw_v = vw_s[:].rearrange("p (l q c m) -> p l q c m",
                                     l=L, q=NPAIR, c=KC)
            uw_v = uw_s[:].rearrange("p (l m n w) -> p l m n w",
                                     l=L, m=KC, n=2)
            cw_v = cw_s[:].rearrange("p (l q m) -> p l q m", l=L, q=NPAIR)

            # per-task state
            vps_t = [None] * T
            cps_t = [None] * T
            vt_t = [None] * T
            cvt_t = [None] * T
            s_sp = [None] * NSP     # s accumulator per span
            xl_sp = [None] * NSP    # current xl tile per span

            def ucv_mm(ti, m):
                sp, l = tasks[ti]
                if l == 0 and m == 0:
                    s_sp[sp] = psum_s.tile([P, KC, NB], F32,
                                           name=f"s_{sp}", tag="s")
                nc.tensor.matmul(
                    s_sp[sp][:, m, :], uw_v[:, tasks[ti][1], m, :, :],
                    cvt_t[ti][:],
                    start=(l == 0 and m % 2 == 0),
                    stop=(l == 0 and m % 2 == 1),
                    perf_mode=DR,
                    skip_group_check=(l > 0 or m % 2 == 1),
                )

            def stt_whole(ti):
                sp, l = tasks[ti]
                cols = slice(sp * NB, (sp + 1) * NB)
                if l < L - 1:
                    xl_sp[sp] = xl_pool.tile([P, KC, NB], BF16,
                                             name=f"xl_{sp}_{l}", tag="xl")
                else:
                    xl_sp[sp] = ot_pool.tile([P, KC, NB], BF16,
                                             name=f"ot_{sp}", tag="ot")
                dst = xl_sp[sp]
                nc.vector.scalar_tensor_tensor(
                    dst[:], s_sp[sp][:], SU, xq_s[:, :, cols], ADD, MULT)
                if l == L - 1:
                    nc.sync.dma_start(out_v[:, sp, :, :], dst[:])

            def cv_mm(ti, q):
                sp, l = tasks[ti]
                nc.tensor.matmul(cps_t[ti][:, q, :], cw_v[:, l, q, :],
                                 vt_t[ti][:, q, :],
                                 start=(q == 0), stop=(q == NPAIR - 1),
                                 skip_group_check=True)

            def v_mm(ti, q, c):
                sp, l = tasks[ti]
                cols = slice(sp * NB, (sp + 1) * NB)
                rhs = xq_s[:, c, cols] if l == 0 else xl_sp[sp][:, c, :]
                nc.tensor.matmul(vps_t[ti][:, q, :], vw_v[:, l, q, c, :],
                                 rhs, start=(c == 0), stop=(c == KC - 1))

            # Per position i (v6-timed): position opens with ucv(t2) DR
            # matmuls (the only start-ready work), with two early v(t)
            # matmuls slotted between them so the ~186ns DR LDWEIGHTS of
            # m2/m3 hide behind real work.  cv(t1) sits ~0.8us in, exactly
            # when vt(t1) lands; stt stays whole (1.1us of slack measured).
            for i in range(T + 2):
                t2, t1, t0 = i - 2, i - 1, i
                if t0 < T:
                    vps_t[t0] = psum_t.tile([P, NPAIR, NB], F32,
                                            name=f"vps_{t0}", tag="t")
                if 0 <= t1 < T:
                    cps_t[t1] = psum_t.tile([P, NPAIR, NB], F32,
                                            name=f"cps_{t1}", tag="t")
                early_v = False
                if t2 >= 0:
                    ucv_mm(t2, 0)
                    ucv_mm(t2, 1)
                    ucv_mm(t2, 2)
                    ucv_mm(t2, 3)
                    stt_whole(t2)
                if 0 <= t1 < T:
                    cv_mm(t1, 0)
                    cv_mm(t1, 1)
                    cvt = cvt_pool.tile([P, NPAIR, NB], F8E4,
                                        name=f"cvt_{t1}", tag="cvt")
                    cvt_t[t1] = cvt
                    nc.scalar.activation(cvt[:], cps_t[t1][:], Tanh)
                if t0 < T:
                    if not early_v:
                        v_mm(t0, 0, 0)
                        v_mm(t0, 0, 1)
                    v_mm(t0, 0, 2)
                    v_mm(t0, 0, 3)
                    for c in range(KC):
                        v_mm(t0, 1, c)
                    vt = vt_pool.tile([P, NPAIR, NB], BF16,
                                      name=f"vt_{t0}", tag="vt")
                    vt_t[t0] = vt
                    nc.scalar.activation(vt[:], vps_t[t0][:], Tanh)

    nc.compile()
    return nc


def _prep_weights(U, V, C):
    VwH = np.empty([P, L, NPAIR, KC, P], dtype=bf16)
    UwH = np.empty([P, L, KC, 2, P], dtype=f8e4)
    CwH = np.zeros([P, L, NPAIR, P], dtype=bf16)
    for l in range(L):
        vscale = 1.0 if l == 0 else 1.0 / SU
        for q in range(NPAIR):
            vpair = np.concatenate([V[l, 2 * q], V[l, 2 * q + 1]],
                                   axis=1) * vscale               # [D, 128]
            for c in range(KC):
                VwH[:, l, q, c, :] = vpair[c * P:(c + 1) * P, :].astype(bf16)
            CwH[:R, l, q, :R] = C[l, 2 * q]
            CwH[R:, l, q, R:] = C[l, 2 * q + 1]
        for i in range(2):   # pair index as DoubleRow plane
            upair = np.concatenate([U[l, 2 * i].T, U[l, 2 * i + 1].T],
                                   axis=0) * SU                   # [128, D]
            for m in range(KC):
                UwH[:, l, m, i, :] = upair[:, m * P:(m + 1) * P].astype(f8e4)
    return (np.ascontiguousarray(VwH.reshape(P, VW_COLS)),
            np.ascontiguousarray(UwH.reshape(P, UW_COLS)),
            np.ascontiguousarray(CwH.reshape(P, CW_COLS)))


def _make_in_maps(x, U, V, C, G, bias):
    vwH, uwH, cwH = _prep_weights(np.asarray(U, np.float32),
                                  np.asarray(V, np.float32),
                                  np.asarray(C, np.float32))
    xT = np.ascontiguousarray(np.asarray(x, np.float32).T).astype(bf16)
    in_maps = []
    for c in range(NCORES):
        cs = slice(c * BC, (c + 1) * BC)
        in_maps.append({
            "xq": np.ascontiguousarray(xT[:, cs]),
            "vw": vwH, "uw": uwH, "cw": cwH,
        })
    return in_maps


def _run(inputs, trace=False, **kw):
    key = "nc"
    if key not in _CACHE:
        _CACHE[key] = _build_nc()
    nc = _CACHE[key]
    in_maps = _make_in_maps(**inputs)
    res = run_bass_kernel_spmd(nc, in_maps, core_ids=list(range(NCORES)),
                               trace=trace, **kw)
    out = np.empty((B, D), np.float32)
    for c in range(NCORES):
        o = res.results[c]["out_s"]                  # [P, NSP*KC*NB] bf16
        o = o.reshape(P, NSP, KC, NB).astype(np.float32) / SU
        # out[b, d]: d = m*128+p, b = sp*256+nb
        out[c * BC:(c + 1) * BC, :] = (
            o.transpose(1, 3, 2, 0).reshape(BC, D))
    return out, res


def kernel(**inputs) -> np.ndarray:
    out, _ = _run(inputs, trace=False)
    return out


# revision 14
# speedup vs baseline: 1.1707x; 1.1707x over previous
"""Trainium2 Bass kernel for the mixed low-rank-expert DCN-v2 block (nn_DCN_51539607711).

Reference math (L=3 layers, E=4 experts, D=512, R=64, B=16384):
  x_{l+1} = sum_e x0 * (tanh(tanh(x_l V_e) C_e) U_e^T + b) * gate_e + x_l
The gate softmaxes a size-1 axis == 1.0 exactly and bias is zero, so the
recurrence telescopes:  x_{l+1} = x0 * (1 + sum_{i<=l} A_i),
  A_i = sum_e U_e tanh(C_e^T tanh(V_e^T x_i)).

v6 design (HAM-aware rotating pipeline):
 - v/cv stages in bf16 (fp8 without DoubleRow runs at bf16 speed on the PE,
   so fp8 there bought only error).  ucv stage keeps e4m3 DoubleRow
   (K=256 in one pass).  End-to-end numpy-sim rel err 0.0128.
 - Single input tensor xq = x^T bf16.  The xl update is
   xl' = (s + SU) * xq  (= SU * x_{l+1}); the 1/SU is folded into the
   V weights for layers l>0 (V'_l = V_l / SU), and the host divides the
   output by SU.  No separate x0s tensor, no on-device rescales.
 - 3-span rotating pipeline: tasks = (span, layer) over NB=256-column
   spans, blocks [[0,1],[2,3,4],[5,6,7]].  PE issue order per step i:
   ucv(t-2), cv(t-1), v(t) - every stage trails its producer by ~2 tasks
   of PE work, so the PE never stalls on the tanh/xl-update chain and the
   HAM clock gate stays at 2.4 GHz.
 - PSUM: s accumulators [P,KC,NB] = 2 banks x 3 in-flight spans (pool
   bufs=3) + 2 shared transient banks for vps/cps (pool bufs=2) = 8.
 - PE pre-warm: dummy matmuls on a zeroed SBUF tile run during the input
   DMA window so the HAM un-throttles before real work arrives.
 - Input DMAs split across sync (xq column blocks) and gpsimd (weights)
   queues so issue does not serialize; output written span-major
   ([p][span][chunk][col] -> 2KB contiguous lines per partition).

Distribution: pure data-parallel over B across 8 cores, weights replicated,
activations feature-major ([D, B]), zero on-device transposes.
"""

import numpy as np
import ml_dtypes

import concourse.bacc as bacc
import concourse.tile as tile
from concourse import mybir
from concourse.bass_utils import run_bass_kernel_spmd

L, E, D, R, B = 3, 4, 512, 64, 16384
NCORES = 8
BC = B // NCORES          # batch columns per core (2048)
NB = 256                  # span width
P = 128
KC = D // P               # feature chunks (4)
NPAIR = E // 2            # expert pairs (2)
NSP = BC // NB            # spans per core (8)

SU = 64.0                 # U-scale; folded into V (l>0) and host unscale

F32 = mybir.dt.float32
BF16 = mybir.dt.bfloat16
F8E4 = mybir.dt.float8e4
DR = mybir.MatmulPerfMode.DoubleRow
bf16 = ml_dtypes.bfloat16
f8e4 = ml_dtypes.float8_e4m3

VW_COLS = L * NPAIR * KC * P             # l, pair, chunk, m   (bf16)
UW_COLS = L * KC * 2 * P                 # l, m, plane, mm     (e4m3)
CW_COLS = L * NPAIR * P                  # l, pair, m          (bf16)

BLOCKS = [[0, 1], [2, 3, 4], [5, 6, 7]]
N_WARM = 10               # pre-warm dummy matmuls (256 cols each)

_CACHE = {}


def _build_nc(bc=BC):
    nc = bacc.Bacc("TRN2", target_bir_lowering=False, debug=False,
                   num_devices=NCORES)

    xq_d = nc.dram_tensor("xq", [D, bc], BF16, kind="ExternalInput")
    vw_d = nc.dram_tensor("vw", [P, VW_COLS], BF16, kind="ExternalInput")
    uw_d = nc.dram_tensor("uw", [P, UW_COLS], F8E4, kind="ExternalInput")
    cw_d = nc.dram_tensor("cw", [P, CW_COLS], BF16, kind="ExternalInput")
    out_d = nc.dram_tensor("out_s", [P, NSP * KC * NB], BF16,
                           kind="ExternalOutput")

    out_v = out_d[:].rearrange("p (s m b) -> p s m b", s=NSP, m=KC)

    Tanh = mybir.ActivationFunctionType.Tanh
    ADD = mybir.AluOpType.add
    MULT = mybir.AluOpType.mult

    tasks = [(sp, l) for blk in BLOCKS for l in range(L) for sp in blk]
    # rotation order within each block: (sp0,l0)(sp1,l0)..(sp0,l1)..
    tasks = []
    for blk in BLOCKS:
        for l in range(L):
            for sp in blk:
                tasks.append((sp, l))
    T = len(tasks)

    with tile.TileContext(nc) as tc:
        with (
            tc.tile_pool(name="wpool", bufs=1) as wpool,
            tc.tile_pool(name="xpool", bufs=1) as xpool,
            tc.tile_pool(name="xl_pool", bufs=4) as xl_pool,
            tc.tile_pool(name="vt_pool", bufs=3) as vt_pool,
            tc.tile_pool(name="cvt_pool", bufs=3) as cvt_pool,
            tc.tile_pool(name="ot_pool", bufs=2) as ot_pool,
            tc.tile_pool(name="warm_pool", bufs=1) as warm_pool,
            tc.tile_pool(name="psum_s", bufs=3, space="PSUM") as psum_s,
            tc.tile_pool(name="psum_t", bufs=2, space="PSUM") as psum_t,
        ):
            xq_s = xpool.tile([P, KC, bc], BF16)
            vw_s = wpool.tile([P, VW_COLS], BF16)
            uw_s = wpool.tile([P, UW_COLS], F8E4)
            cw_s = wpool.tile([P, CW_COLS], BF16)

            xq_v = xq_d[:].rearrange("(k p) b -> p k b", p=P)

            # ---- PE pre-warm: dummy matmuls on zeroed SBUF, no DMA deps.
            warm_w = warm_pool.tile([P, NB], BF16)
            nc.gpsimd.memset(warm_w[:], 0.0)
            warm_ps = psum_t.tile([P, NPAIR, NB], F32, name="warm", tag="t")
            for _ in range(N_WARM):
                nc.tensor.matmul(warm_ps[:, 0, :], warm_w[:, 0:P], warm_w[:],
                                 start=True, stop=True,
                                 skip_group_check=True)

            # ---- input DMAs: weights early on gpsimd (vw_l0 first, then the
            # small cw, then uw, then vw_l12); xq column blocks on sync.
            LW = VW_COLS // L
            XB = 512
            nc.sync.dma_start(xq_s[:, :, 0:XB], xq_v[:, :, 0:XB])
            nc.gpsimd.dma_start(vw_s[:, 0:LW], vw_d[:, 0:LW])
            nc.gpsimd.dma_start(cw_s[:], cw_d[:])
            nc.gpsimd.dma_start(uw_s[:], uw_d[:])
            nc.gpsimd.dma_start(vw_s[:, LW:], vw_d[:, LW:])
            for i in range(1, bc // XB):
                nc.sync.dma_start(xq_s[:, :, i * XB:(i + 1) * XB],
                                  xq_v[:, :, i * XB:(i + 1) * XB])

            vw_v = vw_s[:].rearrange("p (l q c m) -> p l q c m",
                                     l=L, q=NPAIR, c=KC)
            uw_v = uw_s[:].rearrange("p (l m n w) -> p l m n w",
                                     l=L, m=KC, n=2)
            cw_v = cw_s[:].rearrange("p (l q m) -> p l q m", l=L, q=NPAIR)

            # per-task state
            vps_t = [None] * T
            cps_t = [None] * T
            vt_t = [None] * T
            cvt_t = [None] * T
            s_sp = [None] * NSP     # s accumulator per span
            xl_sp = [None] * NSP    # current xl tile per span

            def ucv_mm(ti, m):
                sp, l = tasks[ti]
                if l == 0 and m == 0:
                    s_sp[sp] = psum_s.tile([P, KC, NB], F32,
                                           name=f"s_{sp}", tag="s")
                nc.tensor.matmul(
                    s_sp[sp][:, m, :], uw_v[:, tasks[ti][1], m, :, :],
                    cvt_t[ti][:],
                    start=(l == 0 and m % 2 == 0),
                    stop=(l == 0 and m % 2 == 1),
                    perf_mode=DR,
                    skip_group_check=(l > 0 or m % 2 == 1),
                )

            def stt_whole(ti):
                sp, l = tasks[ti]
                cols = slice(sp * NB, (sp + 1) * NB)
                if l < L - 1:
                    xl_sp[sp] = xl_pool.tile([P, KC, NB], BF16,
                                             name=f"xl_{sp}_{l}", tag="xl")
                else:
                    xl_sp[sp] = ot_pool.tile([P, KC, NB], BF16,
                                             name=f"ot_{sp}", tag="ot")
                dst = xl_sp[sp]
                nc.vector.scalar_tensor_tensor(
                    dst[:], s_sp[sp][:], SU, xq_s[:, :, cols], ADD, MULT)
                if l == L - 1:
                    nc.sync.dma_start(out_v[:, sp, :, :], dst[:])

            def cv_mm(ti, q):
                sp, l = tasks[ti]
                nc.tensor.matmul(cps_t[ti][:, q, :], cw_v[:, l, q, :],
                                 vt_t[ti][:, q, :],
                                 start=(q == 0), stop=(q == NPAIR - 1),
                                 skip_group_check=True)

            def v_mm(ti, q, c):
                sp, l = tasks[ti]
                cols = slice(sp * NB, (sp + 1) * NB)
                rhs = xq_s[:, c, cols] if l == 0 else xl_sp[sp][:, c, :]
                nc.tensor.matmul(vps_t[ti][:, q, :], vw_v[:, l, q, c, :],
                                 rhs, start=(c == 0), stop=(c == KC - 1))

            # Per position i (v6-timed): position opens with ucv(t2) DR
            # matmuls (the only start-ready work), with two early v(t)
            # matmuls slotted between them so the ~186ns DR LDWEIGHTS of
            # m2/m3 hide behind real work.  cv(t1) sits ~0.8us in, exactly
            # when vt(t1) lands; stt stays whole (1.1us of slack measured).
            for i in range(T + 2):
                t2, t1, t0 = i - 2, i - 1, i
                early_v = False
                if t2 >= 0:
                    ucv_mm(t2, 0)
                    ucv_mm(t2, 1)
                    ucv_mm(t2, 2)
                    ucv_mm(t2, 3)
                    stt_whole(t2)
                if 0 <= t1 < T:
                    cps_t[t1] = psum_t.tile([P, NPAIR, NB], F32,
                                            name=f"cps_{t1}", tag="t")
                    cv_mm(t1, 0)
                    cv_mm(t1, 1)
                    cvt = cvt_pool.tile([P, NPAIR, NB], F8E4,
                                        name=f"cvt_{t1}", tag="cvt")
                    cvt_t[t1] = cvt
                    nc.scalar.activation(cvt[:], cps_t[t1][:], Tanh)
                if t0 < T:
                    vps_t[t0] = psum_t.tile([P, NPAIR, NB], F32,
                                            name=f"vps_{t0}", tag="t")
                    if not early_v:
                        v_mm(t0, 0, 0)
                        v_mm(t0, 0, 1)
                    v_mm(t0, 0, 2)
                    v_mm(t0, 0, 3)
                    for c in range(KC):
                        v_mm(t0, 1, c)
                    vt = vt_pool.tile([P, NPAIR, NB], BF16,
                                      name=f"vt_{t0}", tag="vt")
                    vt_t[t0] = vt
                    nc.scalar.activation(vt[:], vps_t[t0][:], Tanh)

    nc.compile()
    return nc


def _prep_weights(U, V, C):
    VwH = np.empty([P, L, NPAIR, KC, P], dtype=bf16)
    UwH = np.empty([P, L, KC, 2, P], dtype=f8e4)
    CwH = np.zeros([P, L, NPAIR, P], dtype=bf16)
    for l in range(L):
        vscale = 1.0 if l == 0 else 1.0 / SU
        for q in range(NPAIR):
            vpair = np.concatenate([V[l, 2 * q], V[l, 2 * q + 1]],
                                   axis=1) * vscale               # [D, 128]
            for c in range(KC):
                VwH[:, l, q, c, :] = vpair[c * P:(c + 1) * P, :].astype(bf16)
            CwH[:R, l, q, :R] = C[l, 2 * q]
            CwH[R:, l, q, R:] = C[l, 2 * q + 1]
        for i in range(2):   # pair index as DoubleRow plane
            upair = np.concatenate([U[l, 2 * i].T, U[l, 2 * i + 1].T],
                                   axis=0) * SU                   # [128, D]
            for m in range(KC):
                UwH[:, l, m, i, :] = upair[:, m * P:(m + 1) * P].astype(f8e4)
    return (np.ascontiguousarray(VwH.reshape(P, VW_COLS)),
            np.ascontiguousarray(UwH.reshape(P, UW_COLS)),
            np.ascontiguousarray(CwH.reshape(P, CW_COLS)))


def _make_in_maps(x, U, V, C, G, bias):
    vwH, uwH, cwH = _prep_weights(np.asarray(U, np.float32),
                                  np.asarray(V, np.float32),
                                  np.asarray(C, np.float32))
    xT = np.ascontiguousarray(np.asarray(x, np.float32).T).astype(bf16)
    in_maps = []
    for c in range(NCORES):
        cs = slice(c * BC, (c + 1) * BC)
        in_maps.append({
            "xq": np.ascontiguousarray(xT[:, cs]),
            "vw": vwH, "uw": uwH, "cw": cwH,
        })
    return in_maps


def _run(inputs, trace=False, **kw):
    key = "nc"
    if key not in _CACHE:
        _CACHE[key] = _build_nc()
    nc = _CACHE[key]
    in_maps = _make_in_maps(**inputs)
    res = run_bass_kernel_spmd(nc, in_maps, core_ids=list(range(NCORES)),
                               trace=trace, **kw)
    out = np.empty((B, D), np.float32)
    for c in range(NCORES):
        o = res.results[c]["out_s"]                  # [P, NSP*KC*NB] bf16
        o = o.reshape(P, NSP, KC, NB).astype(np.float32) / SU
        # out[b, d]: d = m*128+p, b = sp*256+nb
        out[c * BC:(c + 1) * BC, :] = (
            o.transpose(1, 3, 2, 0).reshape(BC, D))
    return out, res


def kernel(**inputs) -> np.ndarray:
    out, _ = _run(inputs, trace=False)
    return out
